# revision 1
# baseline (speedup 1.0000x reference)
"""Causal masked-softmax attention-weight kernel for Trainium2 (8 NeuronCores).

Computes, for query/key of shape [B=2, S=2048, H=16, D=64]:
    w = softmax(where(causal_mask, (Q/sqrt(D)) @ K^T, -inf))  -> [B, H, S, S]

Sharding: the 32 (b, h) pairs are split 4-per-core across 8 cores (data
parallel on B, tensor parallel on H). No cross-core communication.

The host pre-transposes Q/K to [heads, D, S] so the device kernel needs no
on-chip transposes: D lands on SBUF partitions, exactly the matmul
contraction layout.  The 128x128 triangular additive mask is supplied as a
tiny host input (avoids any gpsimd work; the Q7 spin-up was on the critical
path).

Per-core Bass/Tile kernel, per head:
  - DMA K^T / Q^T in [64, 512] chunks via HWDGE (line rate), cast f32->f32r
    on DVE (f32r matmuls stream 2x faster than f32).
  - For q-tile i (128 rows): matmul only the causally-needed k range
    (ncols = 128*(i+1), in N=512 chunks) into one PSUM tile [128, 2048],
    add the triangular -1e9 mask on the diagonal 128x128 block (DVE),
    exp (scale=1/8) on ACT with per-row accumulated sums, reciprocal +
    normalize on DVE, DMA the lower-triangle rows to DRAM.  The
    strictly-upper region is never written: the PJRT run path donates
    pre-zeroed output buffers.
"""

import math
from contextlib import ExitStack

import numpy as np

B, S, H, D = 2, 2048, 16, 64
N_CORES = 8
HPC = (B * H) // N_CORES  # heads (b,h pairs) per core
P = 128  # partitions / q-tile rows
NQT = S // P  # q tiles per head
NCH = S // 512  # 512-col chunks per head
MASK_VAL = -1e9

# matmul operand dtype: "f32" (exact, 4 cyc/row), "f32r" (1 cyc/row, reduced
# precision), "bf16"
MM_DTYPE = "f32r"

_compiled = None


def _build(reps=1):
    import concourse.tile as tile
    from concourse import bacc, mybir

    f32 = mybir.dt.float32

    nc = bacc.Bacc(
        "TRN2",
        target_bir_lowering=False,
        debug=False,
        enable_asserts=False,
        num_devices=N_CORES,
    )
    if MM_DTYPE == "f32r":
        mm_dt = mybir.dt.float32r
    elif MM_DTYPE == "bf16":
        mm_dt = mybir.dt.bfloat16
    else:
        mm_dt = f32

    # host supplies pre-transposed [heads, D, S]
    qT_dram = nc.dram_tensor("qT", [HPC, D, S], f32, kind="ExternalInput").ap()
    kT_dram = nc.dram_tensor("kT", [HPC, D, S], f32, kind="ExternalInput").ap()
    cm_dram = nc.dram_tensor("cm", [P, P], f32, kind="ExternalInput").ap()
    out_dram = nc.dram_tensor("out", [HPC, S, S], f32, kind="ExternalOutput").ap()

    with tile.TileContext(nc) as tc, ExitStack() as ctx:
        consts = ctx.enter_context(tc.tile_pool(name="consts", bufs=1))
        ld_pool = ctx.enter_context(tc.tile_pool(name="ld", bufs=6))
        kt_pool = ctx.enter_context(tc.tile_pool(name="kt", bufs=HPC * NCH))
        qt_pool = ctx.enter_context(tc.tile_pool(name="qt", bufs=HPC * NCH))
        p_pool = ctx.enter_context(tc.tile_pool(name="p", bufs=6))
        st_pool = ctx.enter_context(tc.tile_pool(name="st", bufs=8))
        ps_pool = ctx.enter_context(tc.tile_pool(name="ps", bufs=2, space="PSUM"))

        cmask = consts.tile([P, P], dtype=f32)
        nc.sync.dma_start(cmask[:], cm_dram)

        # warm the ACT exp table off the critical path
        warm = st_pool.tile([P, 1], dtype=f32, tag="warm")
        nc.vector.memset(warm[:], 0.0)
        nc.scalar.activation(
            warm[:], warm[:], mybir.ActivationFunctionType.Exp, bias=0.0, scale=1.0
        )

        rep_ctx = tc.For_i(0, reps, 1) if reps > 1 else None
        if rep_ctx is not None:
            ctx.enter_context(rep_ctx)

        # All reads on HWDGE (no SWDGE anywhere: its Q7 ring init imposes a
        # ~13us barrier on the whole DMA pipeline).  f32 -> f32r rounding on
        # DVE.  Loads and casts are interleaved into the tile emission so
        # neither the sync queue nor the DVE FIFO gets head-of-line blocked.
        qv = {}
        kv = {}

        def load_chunk(j, c):
            for src, pool, tag, dst in (
                (qT_dram, qt_pool, "qt", qv),
                (kT_dram, kt_pool, "kt", kv),
            ):
                sl = src[j][:, c * 512 : (c + 1) * 512]
                if mm_dt == f32:
                    t = pool.tile([D, 512], dtype=f32, tag=tag)
                    nc.sync.dma_start(t[:], sl)
                else:
                    raw = ld_pool.tile([D, 512], dtype=f32, tag="ld")
                    nc.sync.dma_start(raw[:], sl)
                    t = pool.tile([D, 512], dtype=mm_dt, tag=tag)
                    nc.vector.tensor_copy(t[:], raw[:])
                dst[(j, c)] = t[:]

        # prefetch plan: (emitting head, tile index) -> (head, chunk) to load
        plan = {}
        for c in range(2):
            load_chunk(0, c)  # chunks 0..1 cover head 0, tiles 0..7
        plan[(0, 0)] = (0, 2)
        plan[(0, 2)] = (0, 3)
        for i in range(4):  # head 1 loads spread over head 0's mid tiles
            plan[(0, 4 + 2 * i)] = (1, i)
        for j in (1, 2):  # heads 2,3 load during heads 1,2
            for i in range(4):
                plan[(j, 2 + 3 * i)] = (j + 1, i)

        for j in range(HPC):
            for i in range(NQT):
                if (j, i) in plan:
                    load_chunk(*plan[(j, i)])
                ncols = (i + 1) * P
                ql = qv[(j, i // 4)][:, (i % 4) * P : (i % 4 + 1) * P]
                ps = ps_pool.tile([P, S], dtype=f32, tag="ps")
                for m in range(math.ceil(ncols / 512)):
                    # f32r wants N>=256 (full-speed mode); f32 pays per
                    # column, so trim the tail matmul to the exact width
                    w = min(512, ncols - m * 512) if mm_dt == f32 else 512
                    nc.tensor.matmul(
                        ps[:, m * 512 : m * 512 + w],
                        ql,
                        kv[(j, m)][:, 0:w],
                        start=True,
                        stop=True,
                    )
                # diagonal 128x128 block: triangular additive mask
                nc.vector.tensor_add(
                    ps[:, i * P : (i + 1) * P], ps[:, i * P : (i + 1) * P], cmask[:]
                )
                p = p_pool.tile([P, S], dtype=f32, tag="p")
                sums = st_pool.tile([P, 1], dtype=f32, tag="sums")
                nc.scalar.activation(
                    p[:, :ncols],
                    ps[:, :ncols],
                    mybir.ActivationFunctionType.Exp,
                    bias=0.0,
                    scale=1.0 / math.sqrt(D),
                    accum_out=sums[:],
                )
                r = st_pool.tile([P, 1], dtype=f32, tag="r")
                nc.vector.reciprocal(r[:], sums[:])
                nc.vector.tensor_scalar_mul(p[:, :ncols], p[:, :ncols], r[:])
                nc.sync.dma_start(
                    out_dram[j, i * P : (i + 1) * P, 0:ncols], p[:, :ncols]
                )

    nc.compile()
    return nc


def _get_compiled():
    global _compiled
    if _compiled is None:
        _compiled = _build()
    return _compiled


def _make_cmask():
    cm = np.zeros((P, P), dtype=np.float32)
    cm[np.triu_indices(P, 1)] = MASK_VAL
    return cm


def _run(query, key, **spmd_kwargs):
    from concourse import bass_utils

    query = np.asarray(query, dtype=np.float32)
    key = np.asarray(key, dtype=np.float32)
    # [B, S, H, D] -> [B*H, D, S]
    qb = np.ascontiguousarray(np.transpose(query, (0, 2, 3, 1)).reshape(B * H, D, S))
    kb = np.ascontiguousarray(np.transpose(key, (0, 2, 3, 1)).reshape(B * H, D, S))
    cm = _make_cmask()
    in_maps = [
        {
            "qT": qb[c * HPC : (c + 1) * HPC],
            "kT": kb[c * HPC : (c + 1) * HPC],
            "cm": cm,
        }
        for c in range(N_CORES)
    ]
    nc = _get_compiled()
    res = bass_utils.run_bass_kernel_spmd(
        nc, in_maps, core_ids=list(range(N_CORES)), **spmd_kwargs
    )
    outs = [r["out"] for r in res.results]
    return np.concatenate(outs, axis=0).reshape(B, H, S, S), res


def kernel(query, key, mask=None):
    """Full-input entry point: query/key [B, S, H, D] f32, mask ignored
    (always the causal tril).  Returns [B, H, S, S] f32."""
    return _run(query, key)[0]

